# revision 20
# baseline (speedup 1.0000x reference)
"""Trainium2 Bass kernel for the AttentionBlock problem.

Computes, per batch element b (one NeuronCore each, 8 total):
    q = x @ Wq.T ; k = x @ Wk.T ; v = x @ Wv.T        # [N, D]
    scores[q_i, k_i] = <q_i, k_i>                      # [N, N]
    attn = softmax(scores, axis=QUERY)                 # normalize over q per k
    out[q_i, :] = sum_k attn[q_i, k_i] * v[k_i, :]

Shapes: B=8, N=2048, D=512.  Sharding: batch over 8 cores, weights replicated.

Math restructure: S = Q K^T = X (Wq^T Wk) X^T, so with M^T = Wk^T Wq
(precomputed on host) only ONE intermediate U^T = M X^T is needed instead
of both Q and K:
    St[k, q] = S[q, k] = sum_d UT[d, k] * XT[d, q],  UT = M @ XT.

Layout strategy (per core):
    host supplies xT [D, N] (= x[b].T), mT [D, D] (= Wk^T Wq), wvT [D, D]
    (= Wv^T), all pre-rounded to fp16 (same 11-bit mantissa as fp32r but
    half the DMA bytes and a slightly faster PE stream).
    UT[d, n] = M @ XT                 (fp16 tiles, fp32 PSUM)
    V[n, o]  = XT^T @ WvT             (fp16, scaled in place by 1/denom later)
    St[k, q] = UT^T-slices @ XT       (PSUM [128, 2048] per k-tile)
    E = exp(St - max_q) (ACT, fused denom via accum_out) -> fp16
        (softmax over the QUERY axis == free axis here)
    O[q, o] = sum_k E[k, q] * Vs[k, o] (fp16 matmuls, fp32 PSUM)

All matmuls are [128c,128]x[128c,512f] fp16 at ~0.57 ns/row sustained
(the PE power-throttles below its 2.4 GHz burst clock under continuous
load); the kernel is PE-bound at ~96%+ occupancy, ~186 us/core steady
state for the 10.7 GFLOP of work.
"""

import sys

for _p in ("/opt/trn_rl_repo", "/root/.axon_site/_ro/trn_rl_repo"):
    if _p not in sys.path:
        sys.path.append(_p)

import numpy as np
import ml_dtypes  # noqa: F401

import concourse.bass as bass
import concourse.mybir as mybir
import concourse.tile as tile
import bass_rust
from concourse import bass_utils

B, N, D = 8, 2048, 512
P = 128
NT = N // P          # 16 tiles of 128 along N
DT = D // P          # 4 tiles of 128 along D
QC = N // 512        # 4 chunks of 512 along the matmul free dim
F32 = mybir.dt.float32
F32R = mybir.dt.float32r
BF16 = mybir.dt.bfloat16
F16 = mybir.dt.float16


EVICT_ALT = True
XCHUNK = True
PSUM_SHARE = False
B_DD_OUTER = False
Y_RING_ALT = True
A_EVICT_ALT = False
# Moving free-dim chunk per phase.  The PE sustains ~0.57 ns/row under
# continuous load (power-throttled from the 2.4 GHz burst clock); free-dim
# width is near-neutral, with 512 measuring ~1% faster than 128 in
# drift-controlled interleaved A/B runs (and 4x fewer instructions).
B_FREE = 512  # score matmuls
C_FREE = 512  # output matmuls
A_FREE = 512  # projection matmuls
REUSE = False  # order loops so consecutive matmuls share the stationary operand
USE_BF16 = False  # bf16 matmul operands (measured: no faster than f16)


def _dt_in():
    return BF16 if USE_BF16 else F16


def _np_in():
    import ml_dtypes as _md

    return _md.bfloat16 if USE_BF16 else np.float16


def _chunks(total: int, width) -> list:
    """(offset, size) chunks covering `total`.  `width` is an int or a
    repeating pattern list, e.g. [256, 128, 128]."""
    if isinstance(width, int):
        pat = [width]
    else:
        pat = list(width)
    out, off, i = [], 0, 0
    while off < total:
        w = min(pat[i % len(pat)], total - off)
        out.append((off, w))
        off += w
        i += 1
    return out


class _TC(tile.TileContext):
    """TileContext whose kernel-tail drain splits its semaphore waits.

    The walrus build in this container rejects TPB_CTRL instructions
    carrying more than one sync wait; the stock drain attaches one wait
    per logical processor.  Emit one SP nop per pending proc instead.
    """

    def _drain_and_barrier(self, tick_clock, wait_clock):
        vals = list(tick_clock.global_clock)
        n = len(vals)
        for i, v in enumerate(vals):
            if v > 0:
                vc = [0] * n
                vc[i] = v
                nop = self.nc.sync.nop(nofuse=True)
                wait_clock.add_sem_waits(
                    nop.ins, bass_rust.ScopedClock({None: bass_rust.VectorClock(vc)})
                )
        self.nc.sync.drain()
        self.nc.all_engine_barrier()
        assert self.sems is not None
        popped = self.nc._tile_sem_poison_stack.pop()
        assert popped is self._sem_poison
        self.nc.clear_and_free_semaphores(list(self.sems.allocated().values()))
        self.nc.all_engine_barrier()


def _split_waits_json(bir_bytes: bytes) -> bytes:
    """Rewrite BIR so no instruction carries more than one sync wait.

    The walrus build available here rejects instructions with multiple
    sync-wait commands ("Too many sync wait commands").  For every
    instruction with k > 1 waits, insert k-1 NoOp instructions on the same
    engine immediately before it, each carrying one of the excess waits.
    """
    import json

    j = json.loads(bir_bytes)
    ctr = 0
    for fn in j.get("functions", []):
        for blk in fn.get("blocks", []):
            new_insts = []
            for inst in blk.get("instructions", []):
                waits = inst.get("sync_info", {}).get("on_wait", [])
                if len(waits) > 1:
                    keep, extra = waits[0], waits[1:]
                    for w in extra:
                        ctr += 1
                        new_insts.append(
                            {
                                "debug": inst.get("debug", 0),
                                "engine": inst["engine"],
                                "ins": [],
                                "name": f"I-wsplit{ctr}",
                                "opcode": "NoOp",
                                "outs": [],
                                "sync_info": {"on_update": [], "on_wait": [w]},
                            }
                        )
                    inst["sync_info"]["on_wait"] = [keep]
                new_insts.append(inst)
            blk["instructions"] = new_insts
    return json.dumps(j).encode()


def build_nc(iters: int = 1, body_mode: str = "full") -> bass.Bass:
    """Build the per-core program.  iters>1 wraps the body in an on-device
    loop (benchmarking only — output is identical every iteration)."""
    nc = bass.Bass("TRN2", target_bir_lowering=False, debug=False)

    # Inputs are pre-rounded to fp16 on the host (same 11-bit mantissa as
    # fp32r, but 2-byte DMA and ~3% faster PE streaming).
    xT = nc.dram_tensor("xT", [D, N], _dt_in(), kind="ExternalInput")
    mT = nc.dram_tensor("mT", [D, D], _dt_in(), kind="ExternalInput")
    wvT = nc.dram_tensor("wvT", [D, D], _dt_in(), kind="ExternalInput")
    y = nc.dram_tensor("y", [N, D], F32, kind="ExternalOutput")

    with _TC(nc) as tc:
        import contextlib

        loop_cm = tc.For_i(0, iters, 1) if iters > 1 else contextlib.nullcontext()
        with loop_cm:
            _body(nc, tc, xT, mT, wvT, y, body_mode)

    _orig_to_json = nc.to_json_bytes

    def _patched_to_json_bytes():
        return _split_waits_json(_orig_to_json())

    nc.to_json_bytes = _patched_to_json_bytes
    return nc


def _body(nc, tc, xT, mT, wvT, y, body_mode="full"):
    with (
        tc.tile_pool(name="xu", bufs=1) as xu_pool,
        tc.tile_pool(name="vpool", bufs=1) as v_pool,
        tc.tile_pool(name="stats", bufs=4) as stat_pool,
        tc.tile_pool(name="ostage", bufs=4) as o_pool,
    ):
        xr = [xu_pool.tile([P, N], _dt_in(), name=f"xr{i}") for i in range(DT)]
        ut = [xu_pool.tile([P, N], _dt_in(), name=f"ut{i}") for i in range(DT)]
        v = [v_pool.tile([P, D], _dt_in(), name=f"v{i}") for i in range(NT)]
        recips = [stat_pool.tile([P, 1], F32, name=f"recip{i}") for i in range(NT)]

        # ---- Phase A: load inputs; UT = M @ XT; V = X @ WvT ----
        with (
            tc.tile_pool(name="win", bufs=1) as w_pool,
            tc.tile_pool(name="psA", bufs=8, space="PSUM") as psA,
        ):
            mt = [w_pool.tile([P, D], _dt_in(), name=f"mt{i}") for i in range(DT)]
            wv = [w_pool.tile([P, D], _dt_in(), name=f"wv{i}") for i in range(DT)]
            for i in range(DT):
                nc.scalar.dma_start(out=mt[i][:], in_=mT[i * P : (i + 1) * P, :])
            # x arrives column-chunk-major so the first UT matmul groups can
            # start after ~1MB instead of the full 4MB.
            if XCHUNK:
                for ck in range(QC):
                    for i in range(DT):
                        eng = nc.sync if i % 2 == 0 else nc.scalar
                        eng.dma_start(
                            out=xr[i][:, ck * 512 : (ck + 1) * 512],
                            in_=xT[i * P : (i + 1) * P, ck * 512 : (ck + 1) * 512],
                        )
            else:
                for i in range(DT):
                    nc.sync.dma_start(out=xr[i][:], in_=xT[i * P : (i + 1) * P, :])
            for i in range(DT):
                nc.sync.dma_start(out=wv[i][:], in_=wvT[i * P : (i + 1) * P, :])

            if body_mode == "Adma":
                nc.sync.dma_start(out=y[0:P, :], in_=xr[0][:, 0 : 2 * D].bitcast(F32))
                return

            # UT[d, n]: out tile [128d, 512n], contract d' (4 accums).
            # ck outer: each column chunk's groups start as soon as that
            # chunk of x has landed.
            for ck in range(QC):
                for dd_out in range(DT):
                    ps = psA.tile([P, 512], F32, name="psA")
                    a_loop = (
                        [(dd, c) for dd in range(DT) for c in _chunks(512, A_FREE)]
                        if REUSE
                        else [(dd, c) for c in _chunks(512, A_FREE) for dd in range(DT)]
                    )
                    for dd, (ao, aw) in a_loop:
                        nc.tensor.matmul(
                            ps[:, ao : ao + aw],
                            mt[dd][:, dd_out * P : (dd_out + 1) * P],
                            xr[dd][:, ck * 512 + ao : ck * 512 + ao + aw],
                            start=(dd == 0),
                            stop=(dd == DT - 1),
                        )
                    dst = ut[dd_out][:, ck * 512 : (ck + 1) * 512]
                    if A_EVICT_ALT and dd_out % 2 == 1:
                        nc.vector.tensor_copy(dst, ps[:])
                    else:
                        nc.scalar.copy(dst, ps[:])

            # V[n,o]: out tile [128n, 512o], contract d
            for nt in range(NT):
                ps = psA.tile([P, 512], F32, name="psA")
                a_loop = (
                    [(dd, c) for dd in range(DT) for c in _chunks(512, A_FREE)]
                    if REUSE
                    else [(dd, c) for c in _chunks(512, A_FREE) for dd in range(DT)]
                )
                for dd, (ao, aw) in a_loop:
                    nc.tensor.matmul(
                        ps[:, ao : ao + aw],
                        xr[dd][:, nt * P : (nt + 1) * P],
                        wv[dd][:, ao : ao + aw],
                        start=(dd == 0),
                        stop=(dd == DT - 1),
                    )
                if A_EVICT_ALT and nt % 2 == 1:
                    nc.scalar.copy(v[nt][:], ps[:])
                else:
                    nc.vector.tensor_copy(v[nt][:], ps[:])

        if body_mode == "A":
            nc.sync.dma_start(out=y[0:P, :], in_=xr[0][:, 0 : 2 * D].bitcast(F32))
            return

        # ---- Phase B: scores + softmax over query axis ----
        # Scores for one k-tile live in two [128, 1024] PSUM tiles so the
        # row-max of the first half runs while the second half's matmuls
        # stream, and slots recycle at half-tile granularity.
        e_pool = tc.alloc_tile_pool(name="epool", bufs=1)
        e = [e_pool.tile([P, N], _dt_in(), name=f"e{i}") for i in range(NT)]

        def emit_B(psB):
            for kk in range(NT):
                half = []
                mx = stat_pool.tile([P, 2], F32, name="mx")
                if B_DD_OUTER:
                    # dd outer: consecutive matmuls share the stationary
                    # operand (ut[dd] k-slice), avoiding redundant loads
                    half = [
                        psB.tile([P, N // 2], F32, name="psBh") for _ in range(2)
                    ]
                    for dd in range(DT):
                        for h in range(2):
                            for qc in range(2):
                                nc.tensor.matmul(
                                    half[h][:, qc * 512 : (qc + 1) * 512],
                                    ut[dd][:, kk * P : (kk + 1) * P],
                                    xr[dd][
                                        :,
                                        (h * 2 + qc) * 512 : (h * 2 + qc + 1) * 512,
                                    ],
                                    start=(dd == 0),
                                    stop=(dd == DT - 1),
                                )
                    for h in range(2):
                        nc.vector.reduce_max(
                            mx[:, h : h + 1], half[h][:], axis=mybir.AxisListType.X
                        )
                elif REUSE:
                    half = [
                        psB.tile([P, N // 2], F32, name="psBh") for _ in range(2)
                    ]
                    for dd in range(DT):
                        for h in range(2):
                            for qo, qw in _chunks(N // 2, B_FREE):
                                nc.tensor.matmul(
                                    half[h][:, qo : qo + qw],
                                    ut[dd][:, kk * P : (kk + 1) * P],
                                    xr[dd][
                                        :, h * (N // 2) + qo : h * (N // 2) + qo + qw
                                    ],
                                    start=(dd == 0),
                                    stop=(dd == DT - 1),
                                )
                    for h in range(2):
                        nc.vector.reduce_max(
                            mx[:, h : h + 1], half[h][:], axis=mybir.AxisListType.X
                        )
                else:
                    for h in range(2):
                        ph = psB.tile([P, N // 2], F32, name="psBh")
                        half.append(ph)
                        for qo, qw in _chunks(N // 2, B_FREE):
                            for dd in range(DT):
                                nc.tensor.matmul(
                                    ph[:, qo : qo + qw],
                                    ut[dd][:, kk * P : (kk + 1) * P],
                                    xr[dd][
                                        :, h * (N // 2) + qo : h * (N // 2) + qo + qw
                                    ],
                                    start=(dd == 0),
                                    stop=(dd == DT - 1),
                                )
                        nc.vector.reduce_max(
                            mx[:, h : h + 1], ph[:], axis=mybir.AxisListType.X
                        )
                if body_mode == "Bmm":
                    nc.vector.tensor_copy(
                        e[kk][:, 0:512].bitcast(F32), half[0][:, 0:256]
                    )
                    continue
                negmax = stat_pool.tile([P, 1], F32, name="negmax")
                nc.vector.reduce_max(
                    negmax[:], mx[:], axis=mybir.AxisListType.X, negate=True
                )
                dh = stat_pool.tile([P, 2], F32, name="dh")
                for h in range(2):
                    nc.scalar.activation(
                        e[kk][:, h * 1024 : (h + 1) * 1024],
                        half[h][:],
                        mybir.ActivationFunctionType.Exp,
                        bias=negmax[:],
                        accum_out=dh[:, h : h + 1],
                    )
                denom = stat_pool.tile([P, 1], F32, name="denom")
                nc.vector.reduce_sum(denom[:], dh[:], axis=mybir.AxisListType.X)
                nc.vector.reciprocal(recips[kk][:], denom[:])
                # scale V rows in place as soon as this k-tile's denom is known
                nc.vector.tensor_scalar_mul(v[kk][:], v[kk][:], recips[kk][:])

        def emit_C(psC):
            for qq in range(NT):
                ps = psC.tile([P, D], F32, name="psC")
                c_loop = (
                    [(kk, c) for kk in range(NT) for c in _chunks(D, C_FREE)]
                    if REUSE
                    else [(kk, c) for c in _chunks(D, C_FREE) for kk in range(NT)]
                )
                for kk, (oo, ow) in c_loop:
                    nc.tensor.matmul(
                        ps[:, oo : oo + ow],
                        e[kk][:, qq * P : (qq + 1) * P],
                        v[kk][:, oo : oo + ow],
                        start=(kk == 0),
                        stop=(kk == NT - 1),
                    )
                if body_mode == "Cmm":
                    continue
                o = o_pool.tile([P, D], F32, name="ostage")
                if EVICT_ALT and qq % 2 == 1:
                    nc.vector.tensor_copy(o[:], ps[:])
                else:
                    nc.scalar.copy(o[:], ps[:])
                out_eng = nc.scalar if (Y_RING_ALT and qq % 2 == 1) else nc.sync
                out_eng.dma_start(out=y[qq * P : (qq + 1) * P, :], in_=o[:])
            if body_mode == "Cmm":
                o = o_pool.tile([P, D], F32, name="ostage")
                nc.scalar.copy(o[:], ps[:])
                nc.sync.dma_start(out=y[0:P, :], in_=o[:])

        if PSUM_SHARE and body_mode == "full":
            # B gets 3 half-tile slots (6 banks), C coexists with 2 banks so
            # its first accumulation chains interleave into B's tail.
            with (
                tc.tile_pool(name="psB", bufs=3, space="PSUM") as psB,
                tc.tile_pool(name="psC", bufs=2, space="PSUM") as psC,
            ):
                emit_B(psB)
                emit_C(psC)
        else:
            with tc.tile_pool(name="psB", bufs=4, space="PSUM") as psB:
                emit_B(psB)
            if body_mode in ("AB", "Bmm"):
                nc.sync.dma_start(
                    out=y[0:P, :], in_=e[0][:, 0:1024].bitcast(F32)
                )
                e_pool.release()
                return
            with tc.tile_pool(name="psC", bufs=8, space="PSUM") as psC:
                emit_C(psC)

        e_pool.release()


_NC_CACHE = None


def _get_nc():
    global _NC_CACHE
    if _NC_CACHE is None:
        _NC_CACHE = build_nc()
    return _NC_CACHE


def _round_f32r(x: np.ndarray) -> np.ndarray:
    """Round fp32 to the fp32r grid (11 explicit mantissa bits, RTNE-ish)."""
    xi = np.ascontiguousarray(x, dtype=np.float32).view(np.uint32).astype(np.uint64)
    xi = ((xi + (1 << 11)) >> 12) << 12
    return xi.astype(np.uint32).view(np.float32)


def kernel(x: np.ndarray, Wq: np.ndarray, Wk: np.ndarray, Wv: np.ndarray, **_kw):
    assert x.shape == (B, N, D), x.shape
    nc = _get_nc()
    wq64 = np.asarray(Wq, dtype=np.float64)
    wk64 = np.asarray(Wk, dtype=np.float64)
    npdt = _np_in()
    mT = (wk64.T @ wq64).astype(npdt)
    wvT = np.ascontiguousarray(np.asarray(Wv, dtype=np.float32).T).astype(npdt)
    in_maps = []
    for b in range(B):
        in_maps.append(
            {
                "xT": np.ascontiguousarray(np.asarray(x[b], np.float32).T).astype(
                    npdt
                ),
                "mT": mT,
                "wvT": wvT,
            }
        )
    res = bass_utils.run_bass_kernel_spmd(nc, in_maps, core_ids=list(range(B)))
    return np.stack([res.results[b]["y"] for b in range(B)], axis=0)

